# revision 1
# baseline (speedup 1.0000x reference)
"""MoE gating network (logits -> softmax -> top-2) on 8 trn2 NeuronCores.

Reference computation (jax):
    logits = einsum("bsd,ed->bse", x, gate_w) + gate_b     # [4, 4096, 64]
    weights = softmax(logits, axis=-1)
    topk_weights, topk_indices = top_k(weights, 2)
    return topk_weights, topk_indices, weights

Sharding: data parallel over tokens. 16384 tokens split into 8 shards of
2048; the tiny gate weight is replicated. Each core computes its shard's
logits on the PE (f32), softmax on ACT/DVE, and top-2 via the DVE
InstMax/InstMaxIndex ops (8 largest per partition, descending).

Host-side prep per shard: x is transposed to [d_model, tokens] so every
device DMA is a wide contiguous read and the matmul operands have the
contraction dim on partitions (the PE contracts over partitions; x in
natural [token, d] layout would need an on-chip transpose of 16.8 MB/core).
"""

import numpy as np

import concourse.bass as bass
import concourse.mybir as mybir
import concourse.tile as tile
from concourse import bacc
from concourse.bass_utils import run_bass_kernel_spmd

# Problem shape (hardcoded per contract; kernel.py must be self-contained).
B, S, D, E = 4, 4096, 2048, 64
NCORES = 8
TOK = B * S                  # 16384 tokens
TPC = TOK // NCORES          # 2048 tokens per core
P = 128                      # partitions
KC = D // P                  # 16 contraction chunks
TBLK = 512                   # token block per x DMA
NBLK = TPC // TBLK           # 4 blocks per core
MT_PER_BLK = TBLK // P       # 4 matmul token-tiles per block
NMT = TPC // P               # 16 token-tiles per core
F32 = mybir.dt.float32
U32 = mybir.dt.uint32

_cache = {}


def _build_program():
    nc = bacc.Bacc(
        "TRN2", target_bir_lowering=False, debug=False, num_devices=NCORES
    )

    xt = nc.dram_tensor("xt", [D, TPC], F32, kind="ExternalInput").ap()
    wtp = nc.dram_tensor("wtp", [P, KC * E], F32, kind="ExternalInput").ap()
    gb = nc.dram_tensor("gb", [P, E], F32, kind="ExternalInput").ap()
    w_out = nc.dram_tensor("w_out", [TPC, E], F32, kind="ExternalOutput").ap()
    tv_out = nc.dram_tensor("tv_out", [TPC, 2], F32, kind="ExternalOutput").ap()
    ti_out = nc.dram_tensor("ti_out", [TPC, 2], U32, kind="ExternalOutput").ap()

    # DRAM views matching on-chip layouts.
    xt_v = xt.rearrange("(c p) t -> p c t", p=P)          # [128, 16, 2048]
    w_out_v = w_out.rearrange("(b i p) e -> b p i e", i=MT_PER_BLK, p=P)
    tv_v = tv_out.rearrange("(j p) k -> p j k", p=P)      # [128, 16, 2]
    ti_v = ti_out.rearrange("(j p) k -> p j k", p=P)

    with tile.TileContext(nc) as tc:
        with (
            tc.tile_pool(name="const", bufs=1) as const_pool,
            tc.tile_pool(name="xin", bufs=3) as x_pool,
            tc.tile_pool(name="psum", bufs=4, space="PSUM") as psum_pool,
            tc.tile_pool(name="sm", bufs=4) as sm_pool,
            tc.tile_pool(name="stat", bufs=8) as stat_pool,
            tc.tile_pool(name="wout", bufs=2) as wout_pool,
            tc.tile_pool(name="top", bufs=1) as top_pool,
        ):
            wt_sb = const_pool.tile([P, KC, E], F32)
            nc.sync.dma_start(out=wt_sb[:], in_=wtp.rearrange("p (c e) -> p c e", c=KC))
            gb_sb = const_pool.tile([P, E], F32)
            nc.sync.dma_start(out=gb_sb[:], in_=gb[:])

            vals_buf = top_pool.tile([P, NMT, 8], F32)
            idx_buf = top_pool.tile([P, NMT, 8], U32)

            for blk in range(NBLK):
                x_t = x_pool.tile([P, KC, TBLK], F32)
                nc.sync.dma_start(
                    out=x_t[:], in_=xt_v[:, :, blk * TBLK:(blk + 1) * TBLK]
                )
                w_blk = wout_pool.tile([P, MT_PER_BLK, E], F32)
                for i in range(MT_PER_BLK):
                    j = blk * MT_PER_BLK + i
                    ps = psum_pool.tile([P, E], F32)
                    for c in range(KC):
                        nc.tensor.matmul(
                            ps[:],
                            x_t[:, c, i * P:(i + 1) * P],
                            wt_sb[:, c, :],
                            start=(c == 0),
                            stop=(c == KC - 1),
                        )
                    logits = sm_pool.tile([P, E], F32, tag="logits")
                    nc.vector.tensor_add(logits[:], ps[:], gb_sb[:])
                    e_t = sm_pool.tile([P, E], F32, tag="exp")
                    ssum = stat_pool.tile([P, 1], F32, tag="sum")
                    nc.scalar.activation(
                        e_t[:], logits[:],
                        mybir.ActivationFunctionType.Exp,
                        accum_out=ssum[:],
                    )
                    rcp = stat_pool.tile([P, 1], F32, tag="rcp")
                    nc.vector.reciprocal(rcp[:], ssum[:])
                    nc.vector.tensor_scalar_mul(w_blk[:, i, :], e_t[:], rcp[:])
                    nc.vector.max(vals_buf[:, j, :], w_blk[:, i, :])
                    nc.vector.max_index(
                        idx_buf[:, j, :], vals_buf[:, j, :], w_blk[:, i, :]
                    )
                nc.gpsimd.dma_start(out=w_out_v[blk], in_=w_blk[:])

            nc.gpsimd.dma_start(out=tv_v[:], in_=vals_buf[:, :, 0:2])
            nc.gpsimd.dma_start(out=ti_v[:], in_=idx_buf[:, :, 0:2])

    nc.compile()
    return nc


def _get_program():
    if "nc" not in _cache:
        _cache["nc"] = _build_program()
    return _cache["nc"]


def _prep_inputs(x, gate_w, gate_b):
    x2d = np.ascontiguousarray(x, dtype=np.float32).reshape(TOK, D)
    # Pack gate weight: wtp[p, c, e] = gate_w[e, c*128 + p]
    wtp = np.ascontiguousarray(
        gate_w.T.reshape(KC, P, E).transpose(1, 0, 2)
    ).reshape(P, KC * E)
    gb = np.ascontiguousarray(np.broadcast_to(gate_b, (P, E)), dtype=np.float32)
    in_maps = []
    for s in range(NCORES):
        shard_t = np.ascontiguousarray(x2d[s * TPC:(s + 1) * TPC].T)
        in_maps.append({"xt": shard_t, "wtp": wtp, "gb": gb})
    return in_maps


def kernel(x, gate_w, gate_b, _trace=False, _trace_kwargs=None):
    nc = _get_program()
    in_maps = _prep_inputs(x, gate_w, gate_b)
    res = run_bass_kernel_spmd(
        nc, in_maps, list(range(NCORES)), trace=_trace,
        **(_trace_kwargs or {}),
    )
    outs = res.results
    topk_w = np.concatenate([outs[s]["tv_out"] for s in range(NCORES)], axis=0)
    topk_i = np.concatenate([outs[s]["ti_out"] for s in range(NCORES)], axis=0)
    weights = np.concatenate([outs[s]["w_out"] for s in range(NCORES)], axis=0)
    out = (
        topk_w.reshape(B, S, 2).astype(np.float32),
        topk_i.reshape(B, S, 2).astype(np.int32),
        weights.reshape(B, S, E).astype(np.float32),
    )
    if _trace:
        return out, res
    return out


# revision 2
# speedup vs baseline: 1.1932x; 1.1932x over previous
"""MoE gating network (logits -> softmax -> top-2) on 8 trn2 NeuronCores.

Reference computation (jax):
    logits = einsum("bsd,ed->bse", x, gate_w) + gate_b     # [4, 4096, 64]
    weights = softmax(logits, axis=-1)
    topk_weights, topk_indices = top_k(weights, 2)
    return topk_weights, topk_indices, weights

Sharding: data parallel over tokens. 16384 tokens split into 8 shards of
2048; the tiny gate weight is replicated. Each core computes its shard's
logits on the PE (f32), softmax on ACT/DVE, and top-2 via the DVE
InstMax/InstMaxIndex ops (8 largest per partition, descending).

Layouts are chosen so every DMA descriptor is a wide contiguous run:
- x is host-packed to [block, partition, k-chunk, token] so each block's
  load is 128 descriptors of 16 KB (d_model lands on partitions, as the
  PE contraction requires, without any on-chip transpose).
- outputs are written in partition-major device layouts and unpermuted on
  the host after the gather.
"""

import numpy as np

import concourse.bass as bass
import concourse.mybir as mybir
import concourse.tile as tile
from concourse import bacc
from concourse.bass_utils import run_bass_kernel_spmd

# Problem shape (hardcoded per contract; kernel.py must be self-contained).
B, S, D, E = 4, 4096, 2048, 64
NCORES = 8
TOK = B * S                  # 16384 tokens
TPC = TOK // NCORES          # 2048 tokens per core
P = 128                      # partitions
KC = D // P                  # 16 contraction chunks
TBLK = 256                   # token block per x DMA
NBLK = TPC // TBLK           # 8 blocks per core
MT_PER_BLK = TBLK // P       # 2 matmul token-tiles per block
NMT = TPC // P               # 16 token-tiles per core
F32 = mybir.dt.float32
U32 = mybir.dt.uint32

_cache = {}


def _build_program():
    nc = bacc.Bacc(
        "TRN2", target_bir_lowering=False, debug=False, num_devices=NCORES
    )

    xt = nc.dram_tensor("xt", [NBLK, P, KC, TBLK], F32, kind="ExternalInput").ap()
    wtp = nc.dram_tensor("wtp", [P, KC * E], F32, kind="ExternalInput").ap()
    gb = nc.dram_tensor("gb", [P, E], F32, kind="ExternalInput").ap()
    # Outputs in partition-major layouts; host unpermutes.
    w_out = nc.dram_tensor("w_out", [P, NMT, E], F32, kind="ExternalOutput").ap()
    tv_out = nc.dram_tensor("tv_out", [P, NMT * 2], F32, kind="ExternalOutput").ap()
    ti_out = nc.dram_tensor("ti_out", [P, NMT * 2], U32, kind="ExternalOutput").ap()

    with tile.TileContext(nc) as tc:
        with (
            tc.tile_pool(name="const", bufs=1) as const_pool,
            tc.tile_pool(name="xin", bufs=4) as x_pool,
            tc.tile_pool(name="psum", bufs=4, space="PSUM") as psum_pool,
            tc.tile_pool(name="sm", bufs=4) as sm_pool,
            tc.tile_pool(name="stat", bufs=8) as stat_pool,
            tc.tile_pool(name="wout", bufs=3) as wout_pool,
            tc.tile_pool(name="top", bufs=1) as top_pool,
        ):
            wt_sb = const_pool.tile([P, KC, E], F32)
            nc.sync.dma_start(out=wt_sb[:], in_=wtp.rearrange("p (c e) -> p c e", c=KC))
            gb_sb = const_pool.tile([P, E], F32)
            nc.sync.dma_start(out=gb_sb[:], in_=gb[:])

            vals_buf = top_pool.tile([P, NMT, 8], F32)
            idx_buf = top_pool.tile([P, NMT, 8], U32)

            for blk in range(NBLK):
                x_t = x_pool.tile([P, KC, TBLK], F32)
                nc.sync.dma_start(out=x_t[:], in_=xt[blk])
                w_blk = wout_pool.tile([P, MT_PER_BLK, E], F32)
                for i in range(MT_PER_BLK):
                    j = blk * MT_PER_BLK + i
                    ps = psum_pool.tile([P, E], F32)
                    for c in range(KC):
                        nc.tensor.matmul(
                            ps[:],
                            x_t[:, c, i * P:(i + 1) * P],
                            wt_sb[:, c, :],
                            start=(c == 0),
                            stop=(c == KC - 1),
                        )
                    logits = sm_pool.tile([P, E], F32, tag="logits")
                    nc.vector.tensor_add(logits[:], ps[:], gb_sb[:])
                    e_t = sm_pool.tile([P, E], F32, tag="exp")
                    ssum = stat_pool.tile([P, 1], F32, tag="sum")
                    nc.scalar.activation(
                        e_t[:], logits[:],
                        mybir.ActivationFunctionType.Exp,
                        accum_out=ssum[:],
                    )
                    rcp = stat_pool.tile([P, 1], F32, tag="rcp")
                    nc.vector.reciprocal(rcp[:], ssum[:])
                    nc.vector.tensor_scalar_mul(w_blk[:, i, :], e_t[:], rcp[:])
                    nc.vector.max(vals_buf[:, j, :], w_blk[:, i, :])
                    nc.vector.max_index(
                        idx_buf[:, j, :], vals_buf[:, j, :], w_blk[:, i, :]
                    )
                # Store this block's softmax weights (ACT's HWDGE ring, so
                # stores don't queue behind the x loads on SP's ring).
                nc.scalar.dma_start(
                    out=w_out[:, blk * MT_PER_BLK:(blk + 1) * MT_PER_BLK, :],
                    in_=w_blk[:],
                )

            # Compact top-2 slices into dense tiles, then two small DMAs.
            tv_t = top_pool.tile([P, NMT * 2], F32)
            nc.vector.tensor_copy(
                tv_t.rearrange("p (j k) -> p j k", j=NMT)[:], vals_buf[:, :, 0:2]
            )
            ti_t = top_pool.tile([P, NMT * 2], U32)
            nc.vector.tensor_copy(
                ti_t.rearrange("p (j k) -> p j k", j=NMT)[:], idx_buf[:, :, 0:2]
            )
            nc.scalar.dma_start(out=tv_out[:], in_=tv_t[:])
            nc.scalar.dma_start(out=ti_out[:], in_=ti_t[:])

    nc.compile()
    return nc


def _get_program():
    if "nc" not in _cache:
        _cache["nc"] = _build_program()
    return _cache["nc"]


def _prep_inputs(x, gate_w, gate_b):
    x2d = np.ascontiguousarray(x, dtype=np.float32).reshape(TOK, D)
    # Pack gate weight: wtp[p, c, e] = gate_w[e, c*128 + p]
    wtp = np.ascontiguousarray(
        gate_w.T.reshape(KC, P, E).transpose(1, 0, 2)
    ).reshape(P, KC * E)
    gb = np.ascontiguousarray(np.broadcast_to(gate_b, (P, E)), dtype=np.float32)
    in_maps = []
    for s in range(NCORES):
        sh = x2d[s * TPC:(s + 1) * TPC]
        # xp[blk, p, c, t] = x[blk*TBLK + t, c*128 + p]
        xp = np.ascontiguousarray(
            sh.reshape(NBLK, TBLK, KC, P).transpose(0, 3, 2, 1)
        )
        in_maps.append({"xt": xp, "wtp": wtp, "gb": gb})
    return in_maps


def kernel(x, gate_w, gate_b, _trace=False, _trace_kwargs=None):
    nc = _get_program()
    in_maps = _prep_inputs(x, gate_w, gate_b)
    res = run_bass_kernel_spmd(
        nc, in_maps, list(range(NCORES)), trace=_trace,
        **(_trace_kwargs or {}),
    )
    outs = res.results

    def unpack(name, width):
        # dev [P, NMT, width] -> shard [TPC, width]; token = j*128 + p
        shards = []
        for s in range(NCORES):
            dev = outs[s][name].reshape(P, NMT, width)
            shards.append(dev.transpose(1, 0, 2).reshape(TPC, width))
        return np.concatenate(shards, axis=0)

    topk_w = unpack("tv_out", 2)
    topk_i = unpack("ti_out", 2)
    weights = unpack("w_out", E)
    out = (
        topk_w.reshape(B, S, 2).astype(np.float32),
        topk_i.reshape(B, S, 2).astype(np.int32),
        weights.reshape(B, S, E).astype(np.float32),
    )
    if _trace:
        return out, res
    return out


# revision 3
# speedup vs baseline: 1.2482x; 1.0461x over previous
"""MoE gating network (logits -> softmax -> top-2) on 8 trn2 NeuronCores.

Reference computation (jax):
    logits = einsum("bsd,ed->bse", x, gate_w) + gate_b     # [4, 4096, 64]
    weights = softmax(logits, axis=-1)
    topk_weights, topk_indices = top_k(weights, 2)
    return topk_weights, topk_indices, weights

Sharding: data parallel over tokens. 16384 tokens split into 8 shards of
2048; the tiny gate weight is replicated. Each core computes its shard's
logits on the PE (f32), softmax on ACT/DVE, and top-2 via the DVE
InstMax/InstMaxIndex ops (8 largest per partition, descending).

Per-core pipeline, 16 blocks of 128 tokens:
- x is host-packed to [block, partition, k-chunk, token] so each block's
  1 MB load is 128 descriptors of 8 KB (d_model lands on partitions, as
  the PE contraction requires, without any on-chip transpose).
- gate_b is folded into the PSUM accumulation as a K=1 matmul of a ones
  row against the bias row, so the ACT exp reads logits straight from
  PSUM (accum_out gives the softmax denominator in the same pass).
- top-2 values and indices are compacted per tile into one packed f32
  tile (indices as exact small floats) and stored once at the end.
- outputs use partition-major device layouts, unpermuted on the host.
"""

import numpy as np

import concourse.bass as bass
import concourse.mybir as mybir
import concourse.tile as tile
from concourse import bacc
from concourse.bass_utils import run_bass_kernel_spmd

# Problem shape (hardcoded per contract; kernel.py must be self-contained).
B, S, D, E = 4, 4096, 2048, 64
NCORES = 8
TOK = B * S                  # 16384 tokens
TPC = TOK // NCORES          # 2048 tokens per core
P = 128                      # partitions
KC = D // P                  # 16 contraction chunks
NMT = TPC // P               # 16 token-tiles (= blocks) per core
F32 = mybir.dt.float32
U32 = mybir.dt.uint32

_cache = {}


def _build_program():
    nc = bacc.Bacc(
        "TRN2", target_bir_lowering=False, debug=False, num_devices=NCORES
    )

    xt = nc.dram_tensor("xt", [NMT, P, KC, P], F32, kind="ExternalInput").ap()
    wtp = nc.dram_tensor("wtp", [P, KC * E], F32, kind="ExternalInput").ap()
    gb = nc.dram_tensor("gb", [1, E], F32, kind="ExternalInput").ap()
    # Outputs in partition-major layouts; host unpermutes.
    w_out = nc.dram_tensor("w_out", [P, NMT, E], F32, kind="ExternalOutput").ap()
    # Packed top-2: [p, j, 0:2] = values, [p, j, 2:4] = indices (as floats).
    tvi_out = nc.dram_tensor("tvi_out", [P, NMT * 4], F32, kind="ExternalOutput").ap()

    with tile.TileContext(nc) as tc:
        with (
            tc.tile_pool(name="const", bufs=1) as const_pool,
            tc.tile_pool(name="xin", bufs=6) as x_pool,
            tc.tile_pool(name="psum", bufs=4, space="PSUM") as psum_pool,
            tc.tile_pool(name="sm", bufs=4) as sm_pool,
            tc.tile_pool(name="stat", bufs=8) as stat_pool,
            tc.tile_pool(name="wout", bufs=3) as wout_pool,
            tc.tile_pool(name="top", bufs=4) as top_pool,
            tc.tile_pool(name="tvi", bufs=1) as tvi_pool,
        ):
            wt_sb = const_pool.tile([P, KC, E], F32)
            nc.sync.dma_start(out=wt_sb[:], in_=wtp.rearrange("p (c e) -> p c e", c=KC))
            gb_sb = const_pool.tile([1, E], F32)
            nc.sync.dma_start(out=gb_sb[:], in_=gb[:])
            ones_sb = const_pool.tile([1, P], F32)
            nc.vector.memset(ones_sb[:], 1.0)

            tvi_t = tvi_pool.tile([P, NMT, 4], F32)

            w_pair = None
            for j in range(NMT):
                x_t = x_pool.tile([P, KC, P], F32)
                nc.sync.dma_start(out=x_t[:], in_=xt[j])
                if j % 2 == 0:
                    w_pair = wout_pool.tile([P, 2, E], F32)
                ps = psum_pool.tile([P, E], F32)
                # Bias row seeds the accumulation: ps = ones.T @ gate_b.
                nc.tensor.matmul(ps[:], ones_sb[:], gb_sb[:], start=True, stop=False)
                for c in range(KC):
                    nc.tensor.matmul(
                        ps[:],
                        x_t[:, c, :],
                        wt_sb[:, c, :],
                        start=False,
                        stop=(c == KC - 1),
                    )
                e_t = sm_pool.tile([P, E], F32, tag="exp")
                ssum = stat_pool.tile([P, 1], F32, tag="sum")
                nc.scalar.activation(
                    e_t[:], ps[:],
                    mybir.ActivationFunctionType.Exp,
                    accum_out=ssum[:],
                )
                rcp = stat_pool.tile([P, 1], F32, tag="rcp")
                nc.vector.reciprocal(rcp[:], ssum[:])
                nc.vector.tensor_scalar_mul(w_pair[:, j % 2, :], e_t[:], rcp[:])
                vals8 = top_pool.tile([P, 8], F32, tag="vals")
                idx8 = top_pool.tile([P, 8], U32, tag="idx")
                nc.vector.max(vals8[:], w_pair[:, j % 2, :])
                nc.vector.max_index(idx8[:], vals8[:], w_pair[:, j % 2, :])
                nc.vector.tensor_copy(tvi_t[:, j, 0:2], vals8[:, 0:2])
                nc.vector.tensor_copy(tvi_t[:, j, 2:4], idx8[:, 0:2])
                if j % 2 == 1:
                    # ACT's HWDGE ring so stores don't queue behind x loads.
                    nc.scalar.dma_start(
                        out=w_out[:, j - 1:j + 1, :], in_=w_pair[:]
                    )

            nc.scalar.dma_start(
                out=tvi_out.rearrange("p (j k) -> p j k", j=NMT), in_=tvi_t[:]
            )

    nc.compile()
    return nc


def _get_program():
    if "nc" not in _cache:
        _cache["nc"] = _build_program()
    return _cache["nc"]


def _prep_inputs(x, gate_w, gate_b):
    x2d = np.ascontiguousarray(x, dtype=np.float32).reshape(TOK, D)
    # Pack gate weight: wtp[p, c, e] = gate_w[e, c*128 + p]
    wtp = np.ascontiguousarray(
        gate_w.T.reshape(KC, P, E).transpose(1, 0, 2)
    ).reshape(P, KC * E)
    gb = np.ascontiguousarray(gate_b, dtype=np.float32).reshape(1, E)
    in_maps = []
    for s in range(NCORES):
        sh = x2d[s * TPC:(s + 1) * TPC]
        # xp[j, p, c, t] = x[j*128 + t, c*128 + p]
        xp = np.ascontiguousarray(
            sh.reshape(NMT, P, KC, P).transpose(0, 3, 2, 1)
        )
        in_maps.append({"xt": xp, "wtp": wtp, "gb": gb})
    return in_maps


def kernel(x, gate_w, gate_b, _trace=False, _trace_kwargs=None):
    nc = _get_program()
    in_maps = _prep_inputs(x, gate_w, gate_b)
    res = run_bass_kernel_spmd(
        nc, in_maps, list(range(NCORES)), trace=_trace,
        **(_trace_kwargs or {}),
    )
    outs = res.results

    w_shards, tv_shards, ti_shards = [], [], []
    for s in range(NCORES):
        wdev = outs[s]["w_out"].reshape(P, NMT, E)
        w_shards.append(wdev.transpose(1, 0, 2).reshape(TPC, E))
        tvi = outs[s]["tvi_out"].reshape(P, NMT, 4)
        tv_shards.append(tvi[:, :, 0:2].transpose(1, 0, 2).reshape(TPC, 2))
        ti_shards.append(tvi[:, :, 2:4].transpose(1, 0, 2).reshape(TPC, 2))

    topk_w = np.concatenate(tv_shards, axis=0)
    topk_i = np.concatenate(ti_shards, axis=0)
    weights = np.concatenate(w_shards, axis=0)
    out = (
        topk_w.reshape(B, S, 2).astype(np.float32),
        np.rint(topk_i).reshape(B, S, 2).astype(np.int32),
        weights.reshape(B, S, E).astype(np.float32),
    )
    if _trace:
        return out, res
    return out


# revision 6
# speedup vs baseline: 1.2671x; 1.0152x over previous
"""MoE gating network (logits -> softmax -> top-2) on 8 trn2 NeuronCores.

Reference computation (jax):
    logits = einsum("bsd,ed->bse", x, gate_w) + gate_b     # [4, 4096, 64]
    weights = softmax(logits, axis=-1)
    topk_weights, topk_indices = top_k(weights, 2)
    return topk_weights, topk_indices, weights

Sharding: data parallel over tokens. 16384 tokens split into 8 shards of
2048; the tiny gate weight is replicated. Each core computes its shard's
logits on the PE (f32), softmax on ACT/DVE, and top-2 via the DVE
InstMax/InstMaxIndex ops (8 largest per partition, descending).

Per-core pipeline, 16 blocks of 128 tokens:
- x is host-packed to [block, partition, k-chunk, token] so each block's
  1 MB load is 128 descriptors of 8 KB (d_model lands on partitions, as
  the PE contraction requires, without any on-chip transpose).
- gate_b is folded into the PSUM accumulation as a K=1 matmul of a ones
  row against the bias row, so the ACT exp reads logits straight from
  PSUM (accum_out gives the softmax denominator in the same pass).
- top-2 values and indices are compacted per tile into one packed f32
  tile (indices as exact small floats) and stored once at the end.
- outputs use partition-major device layouts, unpermuted on the host.
"""

import numpy as np

import concourse.bass as bass
import concourse.mybir as mybir
import concourse.tile as tile
from concourse import bacc
from concourse.bass_utils import run_bass_kernel_spmd

# Problem shape (hardcoded per contract; kernel.py must be self-contained).
B, S, D, E = 4, 4096, 2048, 64
NCORES = 8
TOK = B * S                  # 16384 tokens
TPC = TOK // NCORES          # 2048 tokens per core
P = 128                      # partitions
KC = D // P                  # 16 contraction chunks
NMT = TPC // P               # 16 token-tiles (= blocks) per core
F32 = mybir.dt.float32
U32 = mybir.dt.uint32

_cache = {}


def _build_program():
    nc = bacc.Bacc(
        "TRN2", target_bir_lowering=False, debug=False, num_devices=NCORES
    )

    xt = nc.dram_tensor("xt", [NMT, P, KC, P], F32, kind="ExternalInput").ap()
    wtp = nc.dram_tensor("wtp", [P, KC * E], F32, kind="ExternalInput").ap()
    gb = nc.dram_tensor("gb", [1, E], F32, kind="ExternalInput").ap()
    # Outputs in partition-major layouts; host unpermutes.
    w_out = nc.dram_tensor("w_out", [P, NMT, E], F32, kind="ExternalOutput").ap()
    # Packed top-2: [p, j, 0:2] = values, [p, j, 2:4] = indices (as floats).
    tvi_out = nc.dram_tensor("tvi_out", [P, NMT * 4], F32, kind="ExternalOutput").ap()

    with tile.TileContext(nc) as tc:
        with (
            tc.tile_pool(name="const", bufs=1) as const_pool,
            tc.tile_pool(name="xin", bufs=6) as x_pool,
            tc.tile_pool(name="psum", bufs=6, space="PSUM") as psum_pool,
            tc.tile_pool(name="sm", bufs=4) as sm_pool,
            tc.tile_pool(name="stat", bufs=8) as stat_pool,
            tc.tile_pool(name="wout", bufs=3) as wout_pool,
            tc.tile_pool(name="top", bufs=4) as top_pool,
            tc.tile_pool(name="tvi", bufs=1) as tvi_pool,
        ):
            wt_sb = const_pool.tile([P, KC, E], F32)
            nc.sync.dma_start(out=wt_sb[:], in_=wtp.rearrange("p (c e) -> p c e", c=KC))
            gb_sb = const_pool.tile([1, E], F32)
            nc.sync.dma_start(out=gb_sb[:], in_=gb[:])
            ones_sb = const_pool.tile([1, P], F32)
            nc.vector.memset(ones_sb[:], 1.0)

            tvi_t = tvi_pool.tile([P, NMT, 4], F32)

            w_pair = None
            for j in range(NMT):
                x_t = x_pool.tile([P, KC, P], F32)
                # Two half-loads so the first 8 contraction matmuls can
                # start while the second half is still in flight.
                nc.sync.dma_start(out=x_t[:, 0:KC // 2, :], in_=xt[j, :, 0:KC // 2, :])
                nc.sync.dma_start(out=x_t[:, KC // 2:, :], in_=xt[j, :, KC // 2:, :])
                if j % 2 == 0:
                    w_pair = wout_pool.tile([P, 2, E], F32)
                ps = psum_pool.tile([P, E], F32)
                # Bias row seeds the accumulation: ps = ones.T @ gate_b.
                nc.tensor.matmul(ps[:], ones_sb[:], gb_sb[:], start=True, stop=False)
                for c in range(KC):
                    nc.tensor.matmul(
                        ps[:],
                        x_t[:, c, :],
                        wt_sb[:, c, :],
                        start=False,
                        stop=(c == KC - 1),
                    )
                e_t = sm_pool.tile([P, E], F32, tag="exp")
                ssum = stat_pool.tile([P, 1], F32, tag="sum")
                nc.scalar.activation(
                    e_t[:], ps[:],
                    mybir.ActivationFunctionType.Exp,
                    accum_out=ssum[:],
                )
                rcp = stat_pool.tile([P, 1], F32, tag="rcp")
                nc.vector.reciprocal(rcp[:], ssum[:])
                nc.vector.tensor_scalar_mul(w_pair[:, j % 2, :], e_t[:], rcp[:])
                vals8 = top_pool.tile([P, 8], F32, tag="vals")
                idx8 = top_pool.tile([P, 8], U32, tag="idx")
                nc.vector.max(vals8[:], w_pair[:, j % 2, :])
                nc.vector.max_index(idx8[:], vals8[:], w_pair[:, j % 2, :])
                nc.vector.tensor_copy(tvi_t[:, j, 0:2], vals8[:, 0:2])
                nc.vector.tensor_copy(tvi_t[:, j, 2:4], idx8[:, 0:2])
                if j % 2 == 1:
                    if j < NMT - 1:
                        # Mid-stream stores go through the otherwise-idle
                        # GpSimd SWDGE ring: ACT's FIFO would head-of-line
                        # block later exp dispatches, SP's would delay loads.
                        nc.gpsimd.dma_start(
                            out=w_out[:, j - 1:j + 1, :], in_=w_pair[:]
                        )
                    else:
                        # Final store: ACT's exps are done, its ring is free
                        # and HWDGE has lower fixed latency than SWDGE.
                        nc.scalar.dma_start(
                            out=w_out[:, j - 1:j + 1, :], in_=w_pair[:]
                        )

            nc.scalar.dma_start(
                out=tvi_out.rearrange("p (j k) -> p j k", j=NMT), in_=tvi_t[:]
            )

    nc.compile()
    return nc


def _get_program():
    if "nc" not in _cache:
        _cache["nc"] = _build_program()
    return _cache["nc"]


def _prep_inputs(x, gate_w, gate_b):
    x2d = np.ascontiguousarray(x, dtype=np.float32).reshape(TOK, D)
    # Pack gate weight: wtp[p, c, e] = gate_w[e, c*128 + p]
    wtp = np.ascontiguousarray(
        gate_w.T.reshape(KC, P, E).transpose(1, 0, 2)
    ).reshape(P, KC * E)
    gb = np.ascontiguousarray(gate_b, dtype=np.float32).reshape(1, E)
    in_maps = []
    for s in range(NCORES):
        sh = x2d[s * TPC:(s + 1) * TPC]
        # xp[j, p, c, t] = x[j*128 + t, c*128 + p]
        xp = np.ascontiguousarray(
            sh.reshape(NMT, P, KC, P).transpose(0, 3, 2, 1)
        )
        in_maps.append({"xt": xp, "wtp": wtp, "gb": gb})
    return in_maps


def kernel(x, gate_w, gate_b, _trace=False, _trace_kwargs=None):
    nc = _get_program()
    in_maps = _prep_inputs(x, gate_w, gate_b)
    res = run_bass_kernel_spmd(
        nc, in_maps, list(range(NCORES)), trace=_trace,
        **(_trace_kwargs or {}),
    )
    outs = res.results

    w_shards, tv_shards, ti_shards = [], [], []
    for s in range(NCORES):
        wdev = outs[s]["w_out"].reshape(P, NMT, E)
        w_shards.append(wdev.transpose(1, 0, 2).reshape(TPC, E))
        tvi = outs[s]["tvi_out"].reshape(P, NMT, 4)
        tv_shards.append(tvi[:, :, 0:2].transpose(1, 0, 2).reshape(TPC, 2))
        ti_shards.append(tvi[:, :, 2:4].transpose(1, 0, 2).reshape(TPC, 2))

    topk_w = np.concatenate(tv_shards, axis=0)
    topk_i = np.concatenate(ti_shards, axis=0)
    weights = np.concatenate(w_shards, axis=0)
    out = (
        topk_w.reshape(B, S, 2).astype(np.float32),
        np.rint(topk_i).reshape(B, S, 2).astype(np.int32),
        weights.reshape(B, S, E).astype(np.float32),
    )
    if _trace:
        return out, res
    return out


# revision 7
# speedup vs baseline: 1.2722x; 1.0040x over previous
"""MoE gating network (logits -> softmax -> top-2) on 8 trn2 NeuronCores.

Reference computation (jax):
    logits = einsum("bsd,ed->bse", x, gate_w) + gate_b     # [4, 4096, 64]
    weights = softmax(logits, axis=-1)
    topk_weights, topk_indices = top_k(weights, 2)
    return topk_weights, topk_indices, weights

Sharding: data parallel over tokens. 16384 tokens split into 8 shards of
2048; the tiny gate weight is replicated. Each core computes its shard's
logits on the PE (f32), softmax on ACT/DVE, and top-2 via the DVE
InstMax/InstMaxIndex ops (8 largest per partition, descending).

Per-core pipeline, 16 blocks of 128 tokens:
- x is host-packed to [block, partition, k-chunk, token] so each block's
  1 MB load is 128 descriptors of 8 KB (d_model lands on partitions, as
  the PE contraction requires, without any on-chip transpose).
- gate_b is folded into the PSUM accumulation as a K=1 matmul of a ones
  row against the bias row, so the ACT exp reads logits straight from
  PSUM (accum_out gives the softmax denominator in the same pass).
- top-2 values and indices are compacted per tile into one packed f32
  tile (indices as exact small floats) and stored once at the end.
- outputs use partition-major device layouts, unpermuted on the host.
"""

import numpy as np

import concourse.bass as bass
import concourse.mybir as mybir
import concourse.tile as tile
from concourse import bacc
from concourse.bass_utils import run_bass_kernel_spmd

# Problem shape (hardcoded per contract; kernel.py must be self-contained).
B, S, D, E = 4, 4096, 2048, 64
NCORES = 8
TOK = B * S                  # 16384 tokens
TPC = TOK // NCORES          # 2048 tokens per core
P = 128                      # partitions
KC = D // P                  # 16 contraction chunks
NMT = TPC // P               # 16 token-tiles (= blocks) per core
F32 = mybir.dt.float32
U32 = mybir.dt.uint32

_cache = {}


def _build_program():
    nc = bacc.Bacc(
        "TRN2", target_bir_lowering=False, debug=False, num_devices=NCORES
    )

    xt = nc.dram_tensor("xt", [NMT, P, KC, P], F32, kind="ExternalInput").ap()
    wtp = nc.dram_tensor("wtp", [P, KC * E], F32, kind="ExternalInput").ap()
    gb = nc.dram_tensor("gb", [1, E], F32, kind="ExternalInput").ap()
    # Outputs in partition-major layouts; host unpermutes.
    w_out = nc.dram_tensor("w_out", [P, NMT, E], F32, kind="ExternalOutput").ap()
    # Packed top-2: [p, j, 0:2] = values, [p, j, 2:4] = indices (as floats).
    tvi_out = nc.dram_tensor("tvi_out", [P, NMT * 4], F32, kind="ExternalOutput").ap()

    with tile.TileContext(nc) as tc:
        with (
            tc.tile_pool(name="const", bufs=1) as const_pool,
            tc.tile_pool(name="xin", bufs=6) as x_pool,
            tc.tile_pool(name="psum", bufs=6, space="PSUM") as psum_pool,
            tc.tile_pool(name="sm", bufs=4) as sm_pool,
            tc.tile_pool(name="stat", bufs=8) as stat_pool,
            tc.tile_pool(name="wout", bufs=3) as wout_pool,
            tc.tile_pool(name="top", bufs=4) as top_pool,
            tc.tile_pool(name="tvi", bufs=1) as tvi_pool,
        ):
            wt_sb = const_pool.tile([P, KC, E], F32)
            nc.sync.dma_start(out=wt_sb[:], in_=wtp.rearrange("p (c e) -> p c e", c=KC))
            gb_sb = const_pool.tile([1, E], F32)
            nc.sync.dma_start(out=gb_sb[:], in_=gb[:])
            ones_sb = const_pool.tile([1, P], F32)
            nc.vector.memset(ones_sb[:], 1.0)

            tvi_t = tvi_pool.tile([P, NMT, 4], F32)

            w_pair = None
            for j in range(NMT):
                x_t = x_pool.tile([P, KC, P], F32)
                # Two half-loads so the first 8 contraction matmuls can
                # start while the second half is still in flight.
                nc.sync.dma_start(out=x_t[:, 0:KC // 2, :], in_=xt[j, :, 0:KC // 2, :])
                nc.sync.dma_start(out=x_t[:, KC // 2:, :], in_=xt[j, :, KC // 2:, :])
                if j % 2 == 0:
                    w_pair = wout_pool.tile([P, 2, E], F32)
                ps = psum_pool.tile([P, E], F32)
                # Bias row seeds the accumulation: ps = ones.T @ gate_b.
                nc.tensor.matmul(ps[:], ones_sb[:], gb_sb[:], start=True, stop=False)
                for c in range(KC):
                    nc.tensor.matmul(
                        ps[:],
                        x_t[:, c, :],
                        wt_sb[:, c, :],
                        start=False,
                        stop=(c == KC - 1),
                    )
                e_t = sm_pool.tile([P, E], F32, tag="exp")
                ssum = stat_pool.tile([P, 1], F32, tag="sum")
                nc.scalar.activation(
                    e_t[:], ps[:],
                    mybir.ActivationFunctionType.Exp,
                    accum_out=ssum[:],
                )
                rcp = stat_pool.tile([P, 1], F32, tag="rcp")
                nc.vector.reciprocal(rcp[:], ssum[:])
                nc.vector.tensor_scalar_mul(w_pair[:, j % 2, :], e_t[:], rcp[:])
                vals8 = top_pool.tile([P, 8], F32, tag="vals")
                idx8 = top_pool.tile([P, 8], U32, tag="idx")
                nc.vector.max(vals8[:], w_pair[:, j % 2, :])
                nc.vector.max_index(idx8[:], vals8[:], w_pair[:, j % 2, :])
                nc.vector.tensor_copy(tvi_t[:, j, 0:2], vals8[:, 0:2])
                nc.vector.tensor_copy(tvi_t[:, j, 2:4], idx8[:, 0:2])
                if j % 2 == 1:
                    if j < NMT - 1:
                        # Mid-stream stores go through the otherwise-idle
                        # GpSimd SWDGE ring: ACT's FIFO would head-of-line
                        # block later exp dispatches, SP's would delay loads.
                        nc.gpsimd.dma_start(
                            out=w_out[:, j - 1:j + 1, :], in_=w_pair[:]
                        )
                    else:
                        # Final store: ACT's exps are done, its ring is free
                        # and HWDGE has lower fixed latency than SWDGE.
                        nc.scalar.dma_start(
                            out=w_out[:, j - 1:j + 1, :], in_=w_pair[:]
                        )

            # SP's ring is idle by now; runs parallel to ACT's last w store.
            nc.sync.dma_start(
                out=tvi_out.rearrange("p (j k) -> p j k", j=NMT), in_=tvi_t[:]
            )

    nc.compile()
    return nc


def _get_program():
    if "nc" not in _cache:
        _cache["nc"] = _build_program()
    return _cache["nc"]


def _prep_inputs(x, gate_w, gate_b):
    x2d = np.ascontiguousarray(x, dtype=np.float32).reshape(TOK, D)
    # Pack gate weight: wtp[p, c, e] = gate_w[e, c*128 + p]
    wtp = np.ascontiguousarray(
        gate_w.T.reshape(KC, P, E).transpose(1, 0, 2)
    ).reshape(P, KC * E)
    gb = np.ascontiguousarray(gate_b, dtype=np.float32).reshape(1, E)
    in_maps = []
    for s in range(NCORES):
        sh = x2d[s * TPC:(s + 1) * TPC]
        # xp[j, p, c, t] = x[j*128 + t, c*128 + p]
        xp = np.ascontiguousarray(
            sh.reshape(NMT, P, KC, P).transpose(0, 3, 2, 1)
        )
        in_maps.append({"xt": xp, "wtp": wtp, "gb": gb})
    return in_maps


def kernel(x, gate_w, gate_b, _trace=False, _trace_kwargs=None):
    nc = _get_program()
    in_maps = _prep_inputs(x, gate_w, gate_b)
    res = run_bass_kernel_spmd(
        nc, in_maps, list(range(NCORES)), trace=_trace,
        **(_trace_kwargs or {}),
    )
    outs = res.results

    w_shards, tv_shards, ti_shards = [], [], []
    for s in range(NCORES):
        wdev = outs[s]["w_out"].reshape(P, NMT, E)
        w_shards.append(wdev.transpose(1, 0, 2).reshape(TPC, E))
        tvi = outs[s]["tvi_out"].reshape(P, NMT, 4)
        tv_shards.append(tvi[:, :, 0:2].transpose(1, 0, 2).reshape(TPC, 2))
        ti_shards.append(tvi[:, :, 2:4].transpose(1, 0, 2).reshape(TPC, 2))

    topk_w = np.concatenate(tv_shards, axis=0)
    topk_i = np.concatenate(ti_shards, axis=0)
    weights = np.concatenate(w_shards, axis=0)
    out = (
        topk_w.reshape(B, S, 2).astype(np.float32),
        np.rint(topk_i).reshape(B, S, 2).astype(np.int32),
        weights.reshape(B, S, E).astype(np.float32),
    )
    if _trace:
        return out, res
    return out
